# revision 19
# baseline (speedup 1.0000x reference)
"""GCN classifier kernel for Trainium2 (Bass/Tile), 8-core SPMD. v2.

Math: per GCN layer, h' = relu(nd * (A^T (ns * h)) @ W + b)
  == relu(sum_e w_e * p[src_e] + b') per dst, with p = h @ W and
  w_e = ns[src_e] * nd[dst_e].

Key tricks vs v1:
- h1 = relu(q1*W0 + b0) is a function of the host-precomputable scalar
  q1 (layer-1 aggregate), so layer-2 messages need NO gather and NO
  table AllGather: msg = relu([q1[src]; 1]^T @ [W0; b0]) built on-device
  by K=2 outer-product matmuls (PE) + grouped ReLU (ScalarE). W1 is
  applied AFTER aggregation (one matmul per dst block).
- All matmul operands fp16 (1 PE cycle/row vs 4 for fp32); gathers and
  the remaining AllGather move half the bytes.
- Layer-3 table AllGather is split into 7 sub-collectives interleaved
  with the L2 block loop (table rows remapped so each lands
  contiguously); collective cost hides behind compute.
- Classifier head applied before the cross-core reduce: AllReduce moves
  [512,10] instead of [512,128].
- S (one-hot scatter matrices) generated per chunk by a single DVE
  tensor_scalar(is_equal, mult) with fp16 in/out (2x/4x DVE mode).

Device pipeline per core (owns 6272 dst nodes = 49 blocks of 128):
  L2: per block: outer-product msgs + relu; aggT += msg^T@S (PE);
      h2T = relu(W1^T aggT + b1); p2 = h2 @ W2 -> slab2_k
      every 7 blocks: sub-AllGather slab2_k -> table2 slice
  L3: dma_gather msgs = table2[src]; agg += S^T@msg (+ ones x b2);
      h3 = relu(agg); r4 += h3^T @ Sg4 (one [128,512] readout matmul)
  head: out_t = (r4_t^T Wc) * invc_t; AllReduce [512,10]; + bc -> out
"""

import sys

sys.path.insert(0, "/opt/trn_rl_repo")

import numpy as np

import concourse.bass as bass
import concourse.mybir as mybir
import concourse.tile as tile
from concourse import bacc, bass_utils

P = 128
N_CORES = 8
N_NODES = 50000
N_EDGES = 800000
HID = 128
N_GRAPHS = 512
N_CLASSES = 10

NPC = 6272          # nodes per core (49 blocks of 128)
BLOCKS = NPC // P   # 49
NSUB = 7            # sub-slabs per core for chunked AllGather
SUBB = BLOCKS // NSUB   # 7 blocks per sub-slab
SUBR = SUBB * P         # 896 rows per sub-slab
NPAD = NPC * N_CORES    # 50176
HALF0 = 32768       # gather window 0: rows [0, 32768)
BASE1 = NPAD - 32768  # 17408; window 1: rows [17408, 50176)
GA = 8              # gather group size in chunks of 128 edges
F32 = mybir.dt.float32
F16 = mybir.dt.float16
I16 = mybir.dt.int16
I32 = mybir.dt.int32


def _remap_rows(n):
    """Node id -> table2 row under the sub-slab-major AllGather layout.

    Sub-AllGather k concatenates core slabs: row = k*SUBR*8 + c*SUBR + r.
    """
    c = n // NPC
    r = n % NPC
    k = r // SUBR
    return k * (SUBR * N_CORES) + c * SUBR + (r % SUBR)


def _prep_graph(src, dst, graph_ids, seq_idx=False):
    """Host-side preprocessing: degrees, q1, per-core edge schedule."""
    src = np.asarray(src).astype(np.int64)
    dst = np.asarray(dst).astype(np.int64)
    graph_ids = np.asarray(graph_ids).astype(np.int64)

    in_deg = np.bincount(dst, minlength=N_NODES).astype(np.float32)
    out_deg = np.bincount(src, minlength=N_NODES).astype(np.float32)
    ns = np.maximum(out_deg, 1.0) ** -0.5
    nd = np.maximum(in_deg, 1.0) ** -0.5
    # layer-1 aggregate: q1 = nd * segsum_dst((in_deg*ns)[src])
    c0 = (in_deg * ns).astype(np.float64)
    t1 = np.bincount(dst, weights=c0[src], minlength=N_NODES)
    q1 = (nd.astype(np.float64) * t1).astype(np.float32)

    w_edge = (ns[src] * nd[dst]).astype(np.float32)
    rsrc = _remap_rows(src)  # gather row in the remapped table

    counts = np.zeros((N_CORES, BLOCKS, 2), np.int64)
    per_core = []
    for c in range(N_CORES):
        base = c * NPC
        m = (dst >= base) & (dst < base + NPC)
        es, ed, ew, eq = rsrc[m], dst[m], w_edge[m], q1[src[m]]
        dloc = ed - base
        blk = dloc >> 7
        # edges with row in [BASE1, HALF0) fit either gather window; assign
        # per block to minimize chunk padding
        half = (es >= HALF0).astype(np.int64)
        over = (es >= BASE1) & (es < HALF0)
        for b in range(BLOCKS):
            mb = blk == b
            n_low = int(np.count_nonzero(mb & (es < BASE1)))
            n_over = int(np.count_nonzero(mb & over))
            n_high = int(np.count_nonzero(mb & (es >= HALF0)))
            cands = {0, n_over}
            k = (-n_low) % P
            while k <= n_over:
                cands.add(k)
                k += P
            best_x, best_cost = 0, 10**9
            for x in sorted(cands):
                cost = -(-(n_low + x) // P) + -(-(n_high + n_over - x) // P)
                if cost < best_cost:
                    best_cost, best_x = cost, x
            if best_x < n_over:
                idxs_over = np.nonzero(mb & over)[0]
                half[idxs_over[best_x:]] = 1
        order = np.lexsort((es, half, blk))
        es, dloc, ew, eq, blk, half = (
            es[order], dloc[order], ew[order], eq[order], blk[order],
            half[order])
        for b in range(BLOCKS):
            mb = blk == b
            counts[c, b, 0] = np.count_nonzero(mb & (half == 0))
            counts[c, b, 1] = np.count_nonzero(mb & (half == 1))
        per_core.append((es, dloc, ew, eq, blk, half))

    K0 = np.maximum(1, np.ceil(counts[:, :, 0] / P).max(axis=0).astype(np.int64))
    K1 = np.ceil(counts[:, :, 1] / P).max(axis=0).astype(np.int64)
    KA = int(K0.sum())
    KB = int(K1.sum())

    core_arrays = []
    for c in range(N_CORES):
        es, dloc, ew, eq, blk, half = per_core[c]
        idxA = np.zeros(KA * P, np.int32)
        dvA = np.zeros(KA * P, np.float32)
        wA = np.zeros(KA * P, np.float32)
        qA = np.zeros(KA * P, np.float32)
        idxB = np.zeros(KB * P, np.int32)
        dvB = np.zeros(KB * P, np.float32)
        wB = np.zeros(KB * P, np.float32)
        qB = np.zeros(KB * P, np.float32)
        offA = 0
        offB = 0
        for b in range(BLOCKS):
            for h, (idxs, dvs, ws, qs, K, off) in enumerate((
                (idxA, dvA, wA, qA, int(K0[b]), offA),
                (idxB, dvB, wB, qB, int(K1[b]), offB),
            )):
                m = (blk == b) & (half == h)
                n = int(np.count_nonzero(m))
                assert n <= K * P
                sl = slice(off, off + n)
                idxs[sl] = es[m] - (0 if h == 0 else BASE1)
                dvs[sl] = (dloc[m] - b * P).astype(np.float32)
                ws[sl] = ew[m]
                qs[sl] = eq[m]
                # padding stays idx=0, dstv=0, w=0 (contributes 0 via S)
            offA += int(K0[b]) * P
            offB += int(K1[b]) * P

        def idx_layout(v):
            # index i -> partition i%16 (replicated x8), column i//16
            if seq_idx:
                v = np.arange(len(v), dtype=np.int64) % 32000
            r = v.astype(np.int16).reshape(-1, 16).T  # [16, L/16]
            return np.tile(r, (8, 1)).copy()  # [128, L/16]

        def col_layout(v):
            return np.ascontiguousarray(v.reshape(-1, P).T)  # [128, K]

        def qone_layout(v):
            # [2, K*P] fp16: row0 = q1[src] per edge, row1 = 1.0
            r = np.stack([v, np.ones_like(v)]).astype(np.float16)
            return np.ascontiguousarray(r)

        base = c * NPC
        own = np.arange(base, base + NPC)
        real = own < N_NODES
        gph = np.full(NPC, -1.0, np.float32)
        gph[real] = graph_ids[own[real]].astype(np.float32)

        core_arrays.append(dict(
            idxA=idx_layout(idxA), idxB=idx_layout(idxB),
            dvA=col_layout(dvA), wA=col_layout(wA),
            dvB=col_layout(dvB), wB=col_layout(wB),
            qoneA=qone_layout(qA), qoneB=qone_layout(qB),
            gphv=np.ascontiguousarray(gph.reshape(BLOCKS, P).T),
        ))

    cnt = np.bincount(graph_ids, minlength=N_GRAPHS).astype(np.float32)
    invc = (1.0 / np.maximum(cnt, 1.0)).reshape(N_GRAPHS // P, P).T  # [128, 4]
    invc = np.ascontiguousarray(invc)

    sched = dict(K0=K0, K1=K1, KA=KA, KB=KB)
    return sched, core_arrays, invc


def build_nc(sched, reps=1, with_coll=True, with_gather=True,
             with_sgen=True, with_compute=True, with_l2=True, with_l3=True,
             msg_bufs=20, sgen_bufs=40,
             hbuf_bufs=10, outer_bufs=3, msg2_bufs=10, s2_bufs=36):
    """Build and compile the 8-core SPMD Bass program."""
    K0, K1, KA, KB = sched["K0"], sched["K1"], sched["KA"], sched["KB"]
    NGT = N_GRAPHS // P  # 4
    KB1 = max(KB, 1)

    nc = bacc.Bacc("TRN2", target_bir_lowering=False, debug=False,
                   num_devices=N_CORES, num_swdge_queues=4)

    def inp(name, shape, dt=F32):
        return nc.dram_tensor(name, list(shape), dt, kind="ExternalInput").ap()

    d_idxA = inp("idxA", [P, KA * 8], I16)
    d_idxB = inp("idxB", [P, KB1 * 8], I16)
    d_dvA = inp("dvA", [P, KA])
    d_wA = inp("wA", [P, KA])
    d_dvB = inp("dvB", [P, KB1])
    d_wB = inp("wB", [P, KB1])
    d_qoneA = inp("qoneA", [2, KA * P], F16)
    d_qoneB = inp("qoneB", [2, KB1 * P], F16)
    d_gph = inp("gphv", [P, BLOCKS])
    d_invc = inp("invc", [P, NGT])
    d_W0b0 = inp("W0b0", [2, HID], F16)
    d_W1 = inp("W1f", [HID, HID], F16)
    d_W2 = inp("W2f", [HID, HID], F16)
    d_Wc = inp("Wcf", [HID, N_CLASSES], F16)
    d_b1c = inp("b1c", [P, 1])
    d_b2r = inp("b2row", [1, HID], F16)
    d_bcr = inp("bcr", [P, N_CLASSES])

    out = nc.dram_tensor("out", [N_GRAPHS, N_CLASSES], F32,
                         kind="ExternalOutput").ap()

    slabs = [nc.dram_tensor(f"slab2_{k}", [SUBR, HID], F16, kind="Internal").ap()
             for k in range(NSUB)]
    table2 = nc.dram_tensor("table2", [NPAD, HID], F16, kind="Internal",
                            addr_space="Shared").ap()
    partial = nc.dram_tensor("partial", [N_GRAPHS, N_CLASSES], F32,
                             kind="Internal").ap()
    summed = nc.dram_tensor("summed", [N_GRAPHS, N_CLASSES], F32,
                            kind="Internal", addr_space="Shared").ap()

    RG = [list(range(N_CORES))]

    # block -> chunk ranges in streams A and B
    offA = np.concatenate([[0], np.cumsum(K0)]).astype(int)
    offB = np.concatenate([[0], np.cumsum(K1)]).astype(int)
    MAXC = int((np.asarray(K0) + np.asarray(K1)).max())

    with tile.TileContext(nc) as tc:
        with tc.tile_pool(name="const", bufs=1) as cp, \
             tc.tile_pool(name="msg", bufs=msg_bufs) as mp, \
             tc.tile_pool(name="sgen", bufs=sgen_bufs) as sp, \
             tc.tile_pool(name="msg2", bufs=msg2_bufs) as mp2, \
             tc.tile_pool(name="sg4", bufs=4) as sgp, \
             tc.tile_pool(name="s2", bufs=s2_bufs) as sp2, \
             tc.tile_pool(name="hbuf", bufs=hbuf_bufs) as hp, \
             tc.tile_pool(name="qblk", bufs=3) as qp, \
             tc.tile_pool(name="agg_ps", bufs=2, space="PSUM") as agg_ps, \
             tc.tile_pool(name="outer_ps", bufs=outer_bufs, space="PSUM") as outer_ps, \
             tc.tile_pool(name="p_ps", bufs=2, space="PSUM") as p_ps, \
             tc.tile_pool(name="r_ps", bufs=1, space="PSUM") as r_ps:

            def load_const(ap_in, shape, dt=F32):
                t = cp.tile(list(shape), dt, tag=ap_in.name)
                nc.sync.dma_start(t[:], ap_in[:])
                return t

            idxA = load_const(d_idxA, [P, KA * 8], I16)
            idxB = load_const(d_idxB, [P, KB1 * 8], I16)
            dvA = load_const(d_dvA, [P, KA])
            wA = load_const(d_wA, [P, KA])
            dvB = load_const(d_dvB, [P, KB1])
            wB = load_const(d_wB, [P, KB1])
# qoneA/B stay in DRAM; streamed per block (SBUF can't hold [2, K*P])
            gph = load_const(d_gph, [P, BLOCKS])
            invc = load_const(d_invc, [P, NGT])
            W0b0 = load_const(d_W0b0, [2, HID], F16)
            W1f = load_const(d_W1, [HID, HID], F16)
            W2f = load_const(d_W2, [HID, HID], F16)
            Wcf = load_const(d_Wc, [HID, N_CLASSES], F16)
            b1c = load_const(d_b1c, [P, 1])
            b2row = load_const(d_b2r, [1, HID], F16)
            bcr = load_const(d_bcr, [P, N_CLASSES])

            iota_i = cp.tile([P, P], I32, tag="iota_i")
            nc.gpsimd.iota(iota_i[:], pattern=[[1, P]], base=0,
                           channel_multiplier=0)
            iota_h = cp.tile([P, P], F16, tag="iota_h")
            nc.vector.tensor_copy(iota_h[:], iota_i[:])
            iotg_i = cp.tile([P, N_GRAPHS], I32, tag="iotg_i")
            nc.gpsimd.iota(iotg_i[:], pattern=[[1, N_GRAPHS]], base=0,
                           channel_multiplier=0)
            iotg_h = cp.tile([P, N_GRAPHS], F16, tag="iotg_h")
            nc.vector.tensor_copy(iotg_h[:], iotg_i[:])
            ones1 = cp.tile([1, P], F16, tag="ones1")
            nc.vector.memset(ones1[:], 1.0)

            RELU = mybir.ActivationFunctionType.Relu
            COPY = mybir.ActivationFunctionType.Copy
            EQ = mybir.AluOpType.is_equal
            MUL = mybir.AluOpType.mult

            # block -> chunk list over both streams
            def block_chunks(b):
                res = []
                for ca in range(offA[b], offA[b + 1]):
                    res.append(("A", ca))
                for cb in range(offB[b], offB[b + 1]):
                    res.append(("B", cb))
                return res

            def sgen(pool, stream, ci, dt=F16, tag="S"):
                """One-hot scatter matrix for chunk ci: S[e, d] =
                (d == dv_e) * w_e. Single DVE op, fp16 in/out."""
                dv, w = (dvA, wA) if stream == "A" else (dvB, wB)
                S = pool.tile([P, P], dt, tag=tag)
                if with_sgen:
                    nc.vector.tensor_scalar(
                        out=S[:], in0=iota_h[:],
                        scalar1=dv[:][:, ci:ci + 1],
                        scalar2=w[:][:, ci:ci + 1],
                        op0=EQ, op1=MUL)
                return S

            def l2_stage(b):
                """Emit outer-product msgs + relus + S-gens for block b.

                Returns state for the deferred agg/tail stage."""
                chunks = block_chunks(b)
                nchunk = len(chunks)
                nA = int(K0[b])
                nB = nchunk - nA
                qblk = qp.tile([2, MAXC * P], F16, tag="qblk")
                nc.sync.dma_start(
                    qblk[:][:, :nA * P],
                    d_qoneA[:, offA[b] * P:offA[b + 1] * P])
                if nB:
                    nc.sync.dma_start(
                        qblk[:][:, nA * P:nchunk * P],
                        d_qoneB[:, offB[b] * P:offB[b + 1] * P])
                msgs = []
                Ss = []
                for g0 in range(0, nchunk, 4):
                    grp = chunks[g0:g0 + 4]
                    ln = len(grp)
                    mp_ps = outer_ps.tile([P, 4 * P], F32, tag="outps")
                    for j in range(ln):
                        k = g0 + j
                        nc.tensor.matmul(
                            out=mp_ps[:][:, j * P:(j + 1) * P],
                            lhsT=qblk[:][:, k * P:(k + 1) * P],
                            rhs=W0b0[:], start=True, stop=True)
                    msg = mp2.tile([P, 4 * P], F16, tag="msg2")
                    nc.scalar.activation(
                        out=msg[:][:, :ln * P], in_=mp_ps[:][:, :ln * P],
                        func=RELU)
                    msgs.append(msg)
                    for j, (stream, ci) in enumerate(grp):
                        Ss.append(sgen(sp2, stream, ci, tag="S2"))
                return b, msgs, Ss, nchunk

            def l2_finish(state):
                """Agg chain + per-block tail for a previously staged block."""
                b, msgs, Ss, nchunk = state
                aggT = agg_ps.tile([P, P], F32, tag="aggps")
                for k in range(nchunk):
                    nc.tensor.matmul(
                        out=aggT[:],
                        lhsT=msgs[k // 4][:][:, (k % 4) * P:(k % 4 + 1) * P],
                        rhs=Ss[k][:],
                        start=(k == 0), stop=(k == nchunk - 1))
                aggT_sb = hp.tile([P, P], F16, tag="aggsb")
                nc.vector.tensor_copy(aggT_sb[:], aggT[:])
                h2_ps = p_ps.tile([P, P], F32, tag="pps")
                nc.tensor.matmul(out=h2_ps[:], lhsT=W1f[:],
                                 rhs=aggT_sb[:], start=True, stop=True)
                h2T = hp.tile([P, P], F16, tag="h2T")
                nc.scalar.activation(out=h2T[:], in_=h2_ps[:],
                                     func=RELU, bias=b1c[:])
                p2_ps = p_ps.tile([P, P], F32, tag="pps")
                nc.tensor.matmul(out=p2_ps[:], lhsT=h2T[:], rhs=W2f[:],
                                 start=True, stop=True)
                p2 = hp.tile([P, P], F16, tag="p2")
                nc.vector.tensor_copy(p2[:], p2_ps[:])
                k, rb = b // SUBB, b % SUBB
                nc.sync.dma_start(slabs[k][rb * P:(rb + 1) * P, :], p2[:])
                if rb == SUBB - 1 and with_coll:
                    nc.gpsimd.collective_compute(
                        "AllGather", mybir.AluOpType.bypass,
                        replica_groups=RG,
                        ins=[slabs[k][:]],
                        outs=[table2[k * SUBR * N_CORES:
                                     (k + 1) * SUBR * N_CORES, :]])

            for rep in range(reps):
                # ---------------- layer 2 (no gather, rank-1 msgs) --------
                # software-pipelined: block b's agg chain is emitted after
                # block b+1's outer/relu/sgen stage so PE never waits on
                # ScalarE relus of the same block
                pend = None
                for b in range(BLOCKS if (with_compute and with_l2) else 0):
                    st = l2_stage(b)
                    if pend is not None:
                        l2_finish(pend)
                    pend = st
                if pend is not None:
                    l2_finish(pend)

                # ---------------- layer 3 + readout ----------------
                def emit_gathers():
                    """Gather groups, block-sorted across streams."""
                    chunk_src = {}
                    groups = []
                    blockA = np.searchsorted(offA[1:], np.arange(KA),
                                             side="right")
                    blockB = np.searchsorted(offB[1:], np.arange(KB1),
                                             side="right")
                    for stream, K, idx_t, blk_of in (
                            ("A", KA, idxA, blockA), ("B", KB, idxB, blockB)):
                        base_ap = (table2[0:HALF0, :] if stream == "A"
                                   else table2[BASE1:NPAD, :])
                        g0 = 0
                        while g0 < K:
                            ln = min(GA, K - g0)
                            groups.append(
                                (int(blk_of[g0]), stream, g0, ln, base_ap,
                                 idx_t))
                            g0 += ln
                    groups.sort(key=lambda g: (g[0], g[1]))
                    for gi, (_fb, stream, g0, ln, base_ap, idx_t) in \
                            enumerate(groups):
                        mt = mp.tile([P, GA * P], F16, tag="msg")
                        out_ap = mt[:][:, :ln * P].rearrange(
                            "p (a b) -> p a b", b=P)
                        if with_gather:
                            nc.gpsimd.dma_gather(
                                out_ap=out_ap, in_ap=base_ap,
                                idxs_ap=idx_t[:][:, g0 * 8:(g0 + ln) * 8],
                                num_idxs=ln * P, num_idxs_reg=ln * P,
                                elem_size=HID, queue_num=gi % 4)
                        for j in range(ln):
                            chunk_src[(stream, g0 + j)] = (mt, j)
                    return chunk_src

                chunk_src = emit_gathers() if with_l3 else {}
                r4 = r_ps.tile([P, N_GRAPHS], F32, tag="rps",
                               name=f"rps_{rep}")
                # per block: pre-emit all S tiles, then the matmul chain;
                # readout matmul deferred one block so PE never waits on
                # the h3 relu
                pend3 = None
                for b in range(BLOCKS if (with_compute and with_l3) else 0):
                    chunks = block_chunks(b)
                    Ss = [sgen(sp, stream, ci, tag="S3")
                          for stream, ci in chunks]
                    Sg4 = sgp.tile([P, N_GRAPHS], F16, tag="Sg4")
                    nc.vector.tensor_scalar(
                        out=Sg4[:], in0=iotg_h[:],
                        scalar1=gph[:][:, b:b + 1], scalar2=None,
                        op0=EQ)
                    agg = agg_ps.tile([P, P], F32, tag="aggps")
                    for j, (stream, ci) in enumerate(chunks):
                        mt, col = chunk_src[(stream, ci)]
                        nc.tensor.matmul(
                            out=agg[:],
                            lhsT=Ss[j][:],
                            rhs=mt[:][:, col * P:(col + 1) * P],
                            start=(j == 0), stop=False)
                    nc.tensor.matmul(out=agg[:], lhsT=ones1[:], rhs=b2row[:],
                                     start=False, stop=True)
                    h3 = hp.tile([P, P], F16, tag="h3")
                    nc.scalar.activation(out=h3[:], in_=agg[:], func=RELU)
                    if pend3 is not None:
                        h3p, Sg4p, bp = pend3
                        nc.tensor.matmul(
                            out=r4[:], lhsT=h3p[:], rhs=Sg4p[:],
                            start=(bp == 0), stop=False)
                    pend3 = (h3, Sg4, b)
                if pend3 is not None:
                    h3p, Sg4p, bp = pend3
                    nc.tensor.matmul(out=r4[:], lhsT=h3p[:], rhs=Sg4p[:],
                                     start=(bp == 0), stop=True)

                # ---------------- head + tiny AllReduce ----------------
                r4sb = hp.tile([P, N_GRAPHS], F16, tag="r4sb")
                if with_compute and with_l3:
                    nc.scalar.activation(out=r4sb[:], in_=r4[:], func=COPY)
                for t in range(NGT if (with_compute and with_l3) else 0):
                    o_ps = p_ps.tile([P, P], F32, tag="pps")
                    nc.tensor.matmul(
                        out=o_ps[:][:, :N_CLASSES],
                        lhsT=r4sb[:][:, t * P:(t + 1) * P],
                        rhs=Wcf[:], start=True, stop=True)
                    osb = hp.tile([P, N_CLASSES], F32, tag="osb")
                    nc.scalar.activation(out=osb[:],
                                         in_=o_ps[:][:, :N_CLASSES],
                                         func=COPY,
                                         scale=invc[:][:, t:t + 1])
                    nc.sync.dma_start(partial[t * P:(t + 1) * P, :], osb[:])

                if with_coll and with_compute and with_l3:
                    nc.gpsimd.collective_compute(
                        "AllReduce", mybir.AluOpType.add, replica_groups=RG,
                        ins=[partial[:]], outs=[summed[:]])

                for t in range(NGT if (with_compute and with_l3) else 0):
                    ld = hp.tile([P, N_CLASSES], F32, tag="ld")
                    nc.sync.dma_start(ld[:], summed[t * P:(t + 1) * P, :])
                    ob = hp.tile([P, N_CLASSES], F32, tag="ob")
                    nc.vector.tensor_tensor(out=ob[:], in0=ld[:], in1=bcr[:],
                                            op=mybir.AluOpType.add)
                    nc.sync.dma_start(out[t * P:(t + 1) * P, :], ob[:])

    nc.compile()
    return nc


def make_in_maps(core_arrays, invc, W0, b0, W1, b1, W2, b2, Wc, bc):
    W0b0 = np.stack([np.asarray(W0, np.float32).reshape(HID),
                     np.asarray(b0, np.float32).reshape(HID)]) \
        .astype(np.float16)
    common = dict(
        invc=np.ascontiguousarray(invc, np.float32),
        W0b0=np.ascontiguousarray(W0b0),
        W1f=np.ascontiguousarray(np.asarray(W1, np.float16)),
        W2f=np.ascontiguousarray(np.asarray(W2, np.float16)),
        Wcf=np.ascontiguousarray(np.asarray(Wc, np.float16)),
        b1c=np.ascontiguousarray(b1, np.float32).reshape(P, 1),
        b2row=np.ascontiguousarray(
            np.asarray(b2, np.float16).reshape(1, HID)),
        bcr=np.ascontiguousarray(np.tile(
            np.asarray(bc, np.float32).reshape(1, N_CLASSES), (P, 1))),
    )
    in_maps = []
    for c in range(N_CORES):
        m = dict(common)
        ca = core_arrays[c]
        for k in ("idxA", "idxB", "dvA", "wA", "dvB", "wB", "qoneA",
                  "qoneB", "gphv"):
            m[k] = ca[k]
        in_maps.append(m)
    return in_maps


_CACHE = {}


def _get_compiled(src, dst, graph_ids):
    import hashlib
    h = hashlib.md5()
    h.update(np.asarray(src).tobytes())
    h.update(np.asarray(dst).tobytes())
    h.update(np.asarray(graph_ids).tobytes())
    key = h.hexdigest()
    if key not in _CACHE:
        sched, core_arrays, invc = _prep_graph(src, dst, graph_ids)
        nc = build_nc(sched)
        _CACHE[key] = (nc, core_arrays, invc)
    return _CACHE[key]


def kernel(W0, b0, W1, b1, W2, b2, Wc, bc, src, dst, graph_ids,
           num_graphs=None, **_ignored):
    nc, core_arrays, invc = _get_compiled(src, dst, graph_ids)
    in_maps = make_in_maps(core_arrays, invc, W0, b0, W1, b1, W2, b2, Wc, bc)
    res = bass_utils.run_bass_kernel_spmd(
        nc, in_maps, core_ids=list(range(N_CORES)))
    return res.results[0]["out"]
